# revision 25
# baseline (speedup 1.0000x reference)
"""Depthwise causal Conv1d (B=4, S=4096, D=2048, K=7) on 8 trn2 NeuronCores.

Strategy: channel (D) tensor parallelism — 256 channels per core, no
cross-device communication. On the host we slice channels and transpose each
shard to channels-major (B, Dsh, S+K-1) with the conv_state prepended along
time, so on-device the time axis is the SBUF free dimension: the 7 causal
taps become free-dim offset reads of one resident tile, and the per-channel
weights/bias become per-partition scalars.

Per (b, 128-channel) strip the time axis is split across two compute
paths working concurrently from the same resident x tile:
  - TensorE (cols [0, 1536)): 7 accumulating diag-matmuls per 512-col chunk
    (lhsT = diag(w_j), rhs = shifted x slice) into PSUM; ScalarE copies
    PSUM->SBUF with the per-channel bias fused via activation(Identity, bias).
  - VectorE (cols [1536, 4096)): acc = x*w0+b (tensor_scalar), then 6
    in-place scalar_tensor_tensor MACs.
ScalarE also issues the output stores (HWDGE); SyncE issues loads.
(GpSimd cannot run TensorScalar-family opcodes on TRN2, so it only exists
as a disabled option; fp32r matmuls would be ~4x faster on TensorE but are
TF32-class precision — rejected to stay fp32-exact.)

Raw bass (no Tile): explicit semaphores, double-buffered across strips.
Output shards are transposed back and concatenated on the host;
new_conv_state is a pure slice of the input.
"""

import os

import numpy as np

B, S, D, K = 4, 4096, 2048, 7
NCORES = 8
DSH = D // NCORES  # 256 channels per core
NGROUPS = DSH // 128  # 2 partition groups of 128 channels
NSTRIPS = B * NGROUPS  # 8 strips of (128, S) per core
ROWS = B * DSH  # 1024 rows of the flattened per-core input
SPAD = S + K - 1  # 4102
NXBUF = int(os.environ.get("KNXBUF", 4))  # x buffering depth
NACC = int(os.environ.get("KNACC", 3))  # acc buffering depth

# Column split per strip (sums to S): TensorE | GpSimd | VectorE
CPE = int(os.environ.get("KCPE", 1536))  # multiple of 512
CPOOL = int(os.environ.get("KCPOOL", 0))
CDVE = S - CPE - CPOOL
NCHUNK = CPE // 512  # PSUM chunks per strip
NPSUMB = int(os.environ.get("KNPSUMB", 8))  # PSUM bank buffers (of 8)

LAST_RESULT = None  # BassKernelResults of the most recent run (for test.py)

_NC_CACHE = {}


def _build_dma_only(rounds=1, nxbuf=3):
    """Loads + immediate stores, no compute — measures the pure DMA floor."""
    import concourse.bass as bass
    import concourse.mybir as mybir

    f32 = mybir.dt.float32
    nc = bass.Bass()
    xpad = nc.declare_dram_parameter("xpad", [ROWS, SPAD], f32, isOutput=False)
    wb = nc.declare_dram_parameter("wb", [128, 8 * NGROUPS], f32, isOutput=False)
    wdiag = nc.declare_dram_parameter(
        "wdiag", [128, NGROUPS * K * 128], f32, isOutput=False
    )
    out = nc.declare_dram_parameter("out", [ROWS, S], f32, isOutput=True)
    total = rounds * NSTRIPS
    with (
        nc.semaphore("load_sem") as load_sem,
        nc.semaphore("store_sem") as store_sem,
        nc.sbuf_tensor("x_sb", [128, nxbuf * SPAD], f32) as x_sb,
    ):
        with nc.Block() as block:

            @block.sync
            def _(sync):
                for t_ in range(total):
                    s_ = t_ % NSTRIPS
                    if t_ >= nxbuf:
                        sync.wait_ge(store_sem, 16 * (t_ - nxbuf + 1))
                    xb = (t_ % nxbuf) * SPAD
                    sync.dma_start(
                        out=x_sb[:, xb : xb + SPAD],
                        in_=xpad[128 * s_ : 128 * (s_ + 1), :],
                    ).then_inc(load_sem, 16)

            @block.scalar
            def _(scalar):
                for t_ in range(total):
                    s_ = t_ % NSTRIPS
                    xb = (t_ % nxbuf) * SPAD
                    scalar.wait_ge(load_sem, 16 * (t_ + 1))
                    scalar.dma_start(
                        out=out[128 * s_ : 128 * (s_ + 1), :],
                        in_=x_sb[:, xb : xb + S],
                    ).then_inc(store_sem, 16)

    return nc


def _build_bass(rounds=1, cpe=None, cpool=None, nxbuf=None, nacc=None,
                lsplit=None, ssplit=None, npsumb=None, dvesplit=None):
    import concourse.bass as bass
    import concourse.mybir as mybir

    CPE = cpe if cpe is not None else globals()["CPE"]
    CPOOL = cpool if cpool is not None else globals()["CPOOL"]
    NXBUF = nxbuf if nxbuf is not None else globals()["NXBUF"]
    NACC = nacc if nacc is not None else globals()["NACC"]
    NPSUMB = npsumb if npsumb is not None else globals()["NPSUMB"]
    CDVE = S - CPE - CPOOL
    NCHUNK = CPE // 512

    f32 = mybir.dt.float32
    mult = mybir.AluOpType.mult
    add = mybir.AluOpType.add
    ident = mybir.ActivationFunctionType.Identity

    nc = bass.Bass()
    xpad = nc.declare_dram_parameter("xpad", [ROWS, SPAD], f32, isOutput=False)
    wb = nc.declare_dram_parameter("wb", [128, 8 * NGROUPS], f32, isOutput=False)
    wdiag = nc.declare_dram_parameter(
        "wdiag", [128, NGROUPS * K * 128], f32, isOutput=False
    )
    out = nc.declare_dram_parameter("out", [ROWS, S], f32, isOutput=True)

    total = rounds * NSTRIPS

    with (
        nc.semaphore("load_sem") as load_sem,
        nc.semaphore("wb_sem") as wb_sem,
        nc.semaphore("wd_sem") as wd_sem,
        nc.semaphore("dve_sem") as dve_sem,
        nc.semaphore("pool_sem") as pool_sem,
        nc.semaphore("mm_sem") as mm_sem,
        nc.semaphore("act_sem") as act_sem,
        nc.semaphore("store_sem") as store_sem,
        nc.sbuf_tensor("wb_sb", [128, 8 * NGROUPS], f32) as wb_sb,
        nc.sbuf_tensor("wd_sb", [128, NGROUPS * K * 128], f32) as wd_sb,
        nc.sbuf_tensor("x_sb", [128, NXBUF * SPAD], f32) as x_sb,
        nc.sbuf_tensor("acc_sb", [128, NACC * S], f32) as acc_sb,
        nc.psum_tensor("ps", [128, NPSUMB * 512], f32) as ps,
    ):
        with nc.Block() as block:

            # Split point of the per-strip load: PE region first, rest second.
            _lsp = lsplit if lsplit is not None else bool(int(os.environ.get("KLSPLIT", "1")))
            LSPLIT = (min(CPE + K - 1, SPAD) if NCHUNK else 0) if _lsp else 0

            @block.sync
            def _(sync):
                sync.dma_start(out=wb_sb[:, :], in_=wb[:, :]).then_inc(wb_sem, 16)
                for t_ in range(total):
                    s_ = t_ % NSTRIPS
                    if t_ >= NXBUF:
                        # x buffer reuse: strip t_-NXBUF fully consumed
                        v = t_ - NXBUF + 1
                        if CDVE:
                            sync.wait_ge(dve_sem, v)
                        if CPOOL:
                            sync.wait_ge(pool_sem, v)
                        if NCHUNK:
                            sync.wait_ge(mm_sem, v * NCHUNK)
                    xb = (t_ % NXBUF) * SPAD
                    if LSPLIT:
                        sync.dma_start(
                            out=x_sb[:, xb : xb + LSPLIT],
                            in_=xpad[128 * s_ : 128 * (s_ + 1), 0:LSPLIT],
                        ).then_inc(load_sem, 16)
                        sync.dma_start(
                            out=x_sb[:, xb + LSPLIT : xb + SPAD],
                            in_=xpad[128 * s_ : 128 * (s_ + 1), LSPLIT:SPAD],
                        ).then_inc(load_sem, 16)
                    else:
                        sync.dma_start(
                            out=x_sb[:, xb : xb + SPAD],
                            in_=xpad[128 * s_ : 128 * (s_ + 1), :],
                        ).then_inc(load_sem, 16)

            LPS = 32 if LSPLIT else 16  # load_sem increments per strip
            _ssp = ssplit if ssplit is not None else bool(int(os.environ.get("KSSPLIT", "1")))
            SSPLIT = bool(NCHUNK and CDVE) and _ssp
            SPS = 32 if SSPLIT else 16  # store_sem increments per strip

            if NCHUNK:

                @block.tensor
                def _(tensor):
                    tensor.wait_ge(wb_sem, 16)
                    tensor.wait_ge(wd_sem, 16)
                    for t_ in range(total):
                        s_ = t_ % NSTRIPS
                        g = s_ % NGROUPS
                        xb = (t_ % NXBUF) * SPAD
                        tensor.wait_ge(load_sem, LPS * t_ + 16)
                        for c_ in range(NCHUNK):
                            ci = t_ * NCHUNK + c_
                            bank = ci % NPSUMB
                            if ci >= NPSUMB:
                                # PSUM bank reuse: ACT copied it out
                                tensor.wait_ge(act_sem, ci - NPSUMB + 1)
                            c0 = 512 * c_
                            for j in range(K):
                                ins = tensor.matmul(
                                    ps[:, 512 * bank : 512 * (bank + 1)],
                                    wd_sb[:, (K * g + j) * 128 : (K * g + j + 1) * 128],
                                    x_sb[:, xb + c0 + j : xb + c0 + j + 512],
                                    start=(j == 0),
                                    stop=(j == K - 1),
                                )
                            ins.then_inc(mm_sem, 1)

            _dvs = dvesplit if dvesplit is not None else bool(
                int(os.environ.get("KDVESPLIT", "0"))
            )

            @block.vector
            def _(vector):
                if not CDVE:
                    return
                vector.wait_ge(wb_sem, 16)
                off = CPE + CPOOL
                for t_ in range(total):
                    s_ = t_ % NSTRIPS
                    g = s_ % NGROUPS
                    xb = (t_ % NXBUF) * SPAD
                    ab = (t_ % NACC) * S
                    vector.wait_ge(load_sem, LPS * (t_ + 1))
                    if t_ >= NACC:
                        vector.wait_ge(store_sem, SPS * (t_ - NACC + 1))
                    # halves: two independent chains interleaved so each op's
                    # pipe drain overlaps the other chain's fill (if the HW
                    # honors dependencies for the drain)
                    if _dvs:
                        h = CDVE // 2
                        parts = [(off, h), (off + h, CDVE - h)]
                    else:
                        parts = [(off, CDVE)]
                    for p0, pw in parts:
                        vector.tensor_scalar(
                            out=acc_sb[:, ab + p0 : ab + p0 + pw],
                            in0=x_sb[:, xb + p0 : xb + p0 + pw],
                            scalar1=wb_sb[:, 8 * g : 8 * g + 1],
                            scalar2=wb_sb[:, 8 * g + 7 : 8 * g + 8],
                            op0=mult,
                            op1=add,
                        )
                    for j in range(1, K):
                        for pi, (p0, pw) in enumerate(parts):
                            ins = vector.scalar_tensor_tensor(
                                out=acc_sb[:, ab + p0 : ab + p0 + pw],
                                in0=x_sb[:, xb + p0 + j : xb + p0 + j + pw],
                                scalar=wb_sb[:, 8 * g + j : 8 * g + j + 1],
                                in1=acc_sb[:, ab + p0 : ab + p0 + pw],
                                op0=mult,
                                op1=add,
                            )
                            if j == K - 1 and pi == len(parts) - 1:
                                ins.then_inc(dve_sem, 1)

            @block.gpsimd
            def _(gpsimd):
                if not CPOOL:
                    return
                gpsimd.wait_ge(wb_sem, 16)
                off = CPE
                for t_ in range(total):
                    s_ = t_ % NSTRIPS
                    g = s_ % NGROUPS
                    xb = (t_ % NXBUF) * SPAD
                    ab = (t_ % NACC) * S
                    gpsimd.wait_ge(load_sem, LPS * (t_ + 1))
                    if t_ >= NACC:
                        gpsimd.wait_ge(store_sem, SPS * (t_ - NACC + 1))
                    gpsimd.tensor_scalar(
                        out=acc_sb[:, ab + off : ab + off + CPOOL],
                        in0=x_sb[:, xb + off : xb + off + CPOOL],
                        scalar1=wb_sb[:, 8 * g : 8 * g + 1],
                        scalar2=wb_sb[:, 8 * g + 7 : 8 * g + 8],
                        op0=mult,
                        op1=add,
                    )
                    for j in range(1, K):
                        ins = gpsimd.scalar_tensor_tensor(
                            out=acc_sb[:, ab + off : ab + off + CPOOL],
                            in0=x_sb[:, xb + off + j : xb + off + j + CPOOL],
                            scalar=wb_sb[:, 8 * g + j : 8 * g + j + 1],
                            in1=acc_sb[:, ab + off : ab + off + CPOOL],
                            op0=mult,
                            op1=add,
                        )
                        if j == K - 1:
                            ins.then_inc(pool_sem, 1)

            @block.scalar
            def _(scalar):
                scalar.dma_start(out=wd_sb[:, :], in_=wdiag[:, :]).then_inc(wd_sem, 16)
                for t_ in range(total):
                    s_ = t_ % NSTRIPS
                    g = s_ % NGROUPS
                    ab = (t_ % NACC) * S
                    if NCHUNK and t_ >= NACC:
                        scalar.wait_ge(store_sem, SPS * (t_ - NACC + 1))
                    for c_ in range(NCHUNK):
                        ci = t_ * NCHUNK + c_
                        bank = ci % NPSUMB
                        scalar.wait_ge(mm_sem, ci + 1)
                        scalar.activation(
                            out=acc_sb[:, ab + 512 * c_ : ab + 512 * (c_ + 1)],
                            in_=ps[:, 512 * bank : 512 * (bank + 1)],
                            func=ident,
                            bias=wb_sb[:, 8 * g + 7 : 8 * g + 8],
                            scale=1.0,
                        ).then_inc(act_sem, 1)
                    if SSPLIT:
                        # PE region ready (ACT program order covers the copies)
                        scalar.dma_start(
                            out=out[128 * s_ : 128 * (s_ + 1), 0:CPE],
                            in_=acc_sb[:, ab : ab + CPE],
                        ).then_inc(store_sem, 16)
                    if CDVE:
                        scalar.wait_ge(dve_sem, t_ + 1)
                    if CPOOL:
                        scalar.wait_ge(pool_sem, t_ + 1)
                    sc0 = CPE if SSPLIT else 0
                    scalar.dma_start(
                        out=out[128 * s_ : 128 * (s_ + 1), sc0:S],
                        in_=acc_sb[:, ab + sc0 : ab + S],
                    ).then_inc(store_sem, 16)

    return nc


def _build_bass_v3(rounds=1, cpe=1536, npsumb=8, lsplit=True, ssplit=True):
    """v3: DVE consumes strip PAIRS (same weight group) per op via 3D paged
    APs — halves DVE instruction count (and per-op DRAIN overhead)."""
    import concourse.bass as bass
    import concourse.mybir as mybir

    f32 = mybir.dt.float32
    mult = mybir.AluOpType.mult
    add = mybir.AluOpType.add
    ident = mybir.ActivationFunctionType.Identity

    CPE = cpe
    CDVE = S - CPE
    NCHUNK = CPE // 512
    NPSUMB = npsumb
    NXB = 4
    NAC = 4

    assert NSTRIPS % 4 == 0

    nc = bass.Bass()
    xpad = nc.declare_dram_parameter("xpad", [ROWS, SPAD], f32, isOutput=False)
    wb = nc.declare_dram_parameter("wb", [128, 8 * NGROUPS], f32, isOutput=False)
    wdiag = nc.declare_dram_parameter(
        "wdiag", [128, NGROUPS * K * 128], f32, isOutput=False
    )
    out = nc.declare_dram_parameter("out", [ROWS, S], f32, isOutput=True)

    total = rounds * NSTRIPS

    def pair_val(t_):
        # dve_sem value after the pair containing global strip t_ completes
        r_, s_ = divmod(t_, NSTRIPS)
        return r_ * (NSTRIPS // 2) + (s_ // 4) * 2 + (s_ % 4) % 2 + 1

    with (
        nc.semaphore("load_sem") as load_sem,
        nc.semaphore("wb_sem") as wb_sem,
        nc.semaphore("wd_sem") as wd_sem,
        nc.semaphore("dve_sem") as dve_sem,
        nc.semaphore("mm_sem") as mm_sem,
        nc.semaphore("act_sem") as act_sem,
        nc.semaphore("store_sem") as store_sem,
        nc.sbuf_tensor("wb_sb", [128, 8 * NGROUPS], f32) as wb_sb,
        nc.sbuf_tensor("wd_sb", [128, NGROUPS * K * 128], f32) as wd_sb,
        nc.sbuf_tensor("x_sb", [128, NXB * SPAD], f32) as x_sb,
        nc.sbuf_tensor("acc_sb", [128, NAC * S], f32) as acc_sb,
        nc.psum_tensor("ps", [128, NPSUMB * 512], f32) as ps,
    ):
        x4 = x_sb[:, :].rearrange("p (s n) -> p s n", s=NXB)
        a4 = acc_sb[:, :].rearrange("p (s n) -> p s n", s=NAC)

        with nc.Block() as block:
            LSPLIT = (min(CPE + K - 1, SPAD) if NCHUNK else 0) if lsplit else 0
            LPS = 32 if LSPLIT else 16
            SSPLIT = bool(NCHUNK and CDVE) and ssplit
            SPS = 32 if SSPLIT else 16

            @block.sync
            def _(sync):
                sync.dma_start(out=wb_sb[:, :], in_=wb[:, :]).then_inc(wb_sem, 16)
                for t_ in range(total):
                    s_ = t_ % NSTRIPS
                    if t_ >= NXB:
                        if CDVE:
                            sync.wait_ge(dve_sem, pair_val(t_ - NXB))
                        if NCHUNK:
                            sync.wait_ge(mm_sem, (t_ - NXB + 1) * NCHUNK)
                    xb = (t_ % NXB) * SPAD
                    if LSPLIT:
                        sync.dma_start(
                            out=x_sb[:, xb : xb + LSPLIT],
                            in_=xpad[128 * s_ : 128 * (s_ + 1), 0:LSPLIT],
                        ).then_inc(load_sem, 16)
                        sync.dma_start(
                            out=x_sb[:, xb + LSPLIT : xb + SPAD],
                            in_=xpad[128 * s_ : 128 * (s_ + 1), LSPLIT:SPAD],
                        ).then_inc(load_sem, 16)
                    else:
                        sync.dma_start(
                            out=x_sb[:, xb : xb + SPAD],
                            in_=xpad[128 * s_ : 128 * (s_ + 1), :],
                        ).then_inc(load_sem, 16)

            if NCHUNK:

                @block.tensor
                def _(tensor):
                    tensor.wait_ge(wb_sem, 16)
                    tensor.wait_ge(wd_sem, 16)
                    for t_ in range(total):
                        s_ = t_ % NSTRIPS
                        g = s_ % NGROUPS
                        xb = (t_ % NXB) * SPAD
                        tensor.wait_ge(load_sem, LPS * t_ + 16)
                        for c_ in range(NCHUNK):
                            ci = t_ * NCHUNK + c_
                            bank = ci % NPSUMB
                            if ci >= NPSUMB:
                                tensor.wait_ge(act_sem, ci - NPSUMB + 1)
                            c0 = 512 * c_
                            for j in range(K):
                                ins = tensor.matmul(
                                    ps[:, 512 * bank : 512 * (bank + 1)],
                                    wd_sb[:, (K * g + j) * 128 : (K * g + j + 1) * 128],
                                    x_sb[:, xb + c0 + j : xb + c0 + j + 512],
                                    start=(j == 0),
                                    stop=(j == K - 1),
                                )
                            ins.then_inc(mm_sem, 1)

            @block.vector
            def _(vector):
                if not CDVE:
                    return
                vector.wait_ge(wb_sem, 16)
                for t0 in range(0, total, 4):
                    for u in (0, 1):
                        a_ = t0 + u  # pair (a_, a_+2); same weight group
                        b_ = a_ + 2
                        g = (a_ % NSTRIPS) % NGROUPS
                        xs = a_ % NXB  # pages xs, xs+2
                        vector.wait_ge(load_sem, LPS * (b_ + 1))
                        if a_ >= NXB:
                            vector.wait_ge(store_sem, SPS * (b_ - NXB + 1))
                        x3 = x4[:, xs :: 2, :]
                        o3 = a4[:, xs :: 2, CPE:S]
                        vector.tensor_scalar(
                            out=o3,
                            in0=x3[:, :, CPE : CPE + CDVE],
                            scalar1=wb_sb[:, 8 * g : 8 * g + 1],
                            scalar2=wb_sb[:, 8 * g + 7 : 8 * g + 8],
                            op0=mult,
                            op1=add,
                        )
                        for j in range(1, K):
                            ins = vector.scalar_tensor_tensor(
                                out=o3,
                                in0=x3[:, :, CPE + j : CPE + j + CDVE],
                                scalar=wb_sb[:, 8 * g + j : 8 * g + j + 1],
                                in1=o3,
                                op0=mult,
                                op1=add,
                            )
                            if j == K - 1:
                                ins.then_inc(dve_sem, 1)

            @block.scalar
            def _(scalar):
                scalar.dma_start(out=wd_sb[:, :], in_=wdiag[:, :]).then_inc(wd_sem, 16)
                for t_ in range(total):
                    s_ = t_ % NSTRIPS
                    g = s_ % NGROUPS
                    ab = (t_ % NAC) * S
                    if NCHUNK and t_ >= NAC:
                        scalar.wait_ge(store_sem, SPS * (t_ - NAC + 1))
                    for c_ in range(NCHUNK):
                        ci = t_ * NCHUNK + c_
                        bank = ci % NPSUMB
                        scalar.wait_ge(mm_sem, ci + 1)
                        scalar.activation(
                            out=acc_sb[:, ab + 512 * c_ : ab + 512 * (c_ + 1)],
                            in_=ps[:, 512 * bank : 512 * (bank + 1)],
                            func=ident,
                            bias=wb_sb[:, 8 * g + 7 : 8 * g + 8],
                            scale=1.0,
                        ).then_inc(act_sem, 1)
                    if SSPLIT:
                        scalar.dma_start(
                            out=out[128 * s_ : 128 * (s_ + 1), 0:CPE],
                            in_=acc_sb[:, ab : ab + CPE],
                        ).then_inc(store_sem, 16)
                    if CDVE:
                        scalar.wait_ge(dve_sem, pair_val(t_))
                    sc0 = CPE if SSPLIT else 0
                    scalar.dma_start(
                        out=out[128 * s_ : 128 * (s_ + 1), sc0:S],
                        in_=acc_sb[:, ab + sc0 : ab + S],
                    ).then_inc(store_sem, 16)

    return nc


def _get_bass():
    if "nc" not in _NC_CACHE:
        if int(os.environ.get("KV3", "0")):
            _NC_CACHE["nc"] = _build_bass_v3()
        else:
            _NC_CACHE["nc"] = _build_bass()
    return _NC_CACHE["nc"]


def _prepare_in_maps(x, conv_state, W, b):
    x = np.asarray(x, dtype=np.float32)
    conv_state = np.asarray(conv_state, dtype=np.float32)
    W = np.asarray(W, dtype=np.float32)
    b = np.asarray(b, dtype=np.float32)

    in_maps = []
    for c in range(NCORES):
        sl = slice(c * DSH, (c + 1) * DSH)
        xt = x[:, :, sl].transpose(0, 2, 1)  # (B, DSH, S)
        xp = np.concatenate([conv_state[:, sl, :], xt], axis=2)  # (B, DSH, SPAD)
        xp = np.ascontiguousarray(xp).reshape(ROWS, SPAD)
        wc = W[sl, 0, :]  # (DSH, K)
        bc = b[sl]  # (DSH,)
        wbm = np.zeros((128, 8 * NGROUPS), dtype=np.float32)
        for g in range(NGROUPS):
            wbm[:, 8 * g : 8 * g + 7] = wc[128 * g : 128 * (g + 1)]
            wbm[:, 8 * g + 7] = bc[128 * g : 128 * (g + 1)]
        wdm = np.zeros((128, NGROUPS * K * 128), dtype=np.float32)
        idx = np.arange(128)
        for g in range(NGROUPS):
            for j in range(K):
                blk = K * g + j
                wdm[idx, blk * 128 + idx] = wc[128 * g : 128 * (g + 1), j]
        in_maps.append({"xpad": xp, "wb": wbm, "wdiag": wdm})
    return in_maps


def kernel(x, conv_state, W, b):
    global LAST_RESULT
    from concourse.bass_utils import run_bass_kernel_spmd

    x = np.asarray(x, dtype=np.float32)
    in_maps = _prepare_in_maps(x, conv_state, W, b)

    nc = _get_bass()
    trace = bool(os.environ.get("KBENCH_TRACE"))
    res = run_bass_kernel_spmd(nc, in_maps, list(range(NCORES)), trace=trace)
    LAST_RESULT = res

    shards = []
    for c in range(NCORES):
        o = res.results[c]["out"].reshape(B, DSH, S).transpose(0, 2, 1)  # (B,S,DSH)
        shards.append(o)
    out = np.concatenate(shards, axis=2)  # (B, S, D)

    new_state = np.ascontiguousarray(x[:, S - (K - 1) :, :].transpose(0, 2, 1))
    return out, new_state


# revision 28
# speedup vs baseline: 1.7250x; 1.7250x over previous
"""Depthwise causal Conv1d (B=4, S=4096, D=2048, K=7) on 8 trn2 NeuronCores.

Strategy: channel (D) tensor parallelism — 256 channels per core, no
cross-device communication. On the host we slice channels and transpose each
shard to channels-major (B, Dsh, S+K-1) with the conv_state prepended along
time, so on-device the time axis is the SBUF free dimension: the 7 causal
taps become free-dim offset reads of one resident tile, and the per-channel
weights/bias become per-partition scalars.

Per (b, 128-channel) strip the time axis is split across two compute
paths working concurrently from the same resident x tile:
  - TensorE (cols [0, 1536)): 7 accumulating diag-matmuls per 512-col chunk
    (lhsT = diag(w_j), rhs = shifted x slice) into PSUM; ScalarE copies
    PSUM->SBUF with the per-channel bias fused via activation(Identity, bias).
  - VectorE (cols [1536, 4096)): acc = x*w0+b (tensor_scalar), then 6
    in-place scalar_tensor_tensor MACs.
ScalarE also issues the output stores (HWDGE); SyncE issues loads.
(GpSimd cannot run TensorScalar-family opcodes on TRN2, so it only exists
as a disabled option; fp32r matmuls would be ~4x faster on TensorE but are
TF32-class precision — rejected to stay fp32-exact.)

Raw bass (no Tile): explicit semaphores, double-buffered across strips.
Output shards are transposed back and concatenated on the host;
new_conv_state is a pure slice of the input.
"""

import os

import numpy as np

B, S, D, K = 4, 4096, 2048, 7
NCORES = 8
DSH = D // NCORES  # 256 channels per core
NGROUPS = DSH // 128  # 2 partition groups of 128 channels
NSTRIPS = B * NGROUPS  # 8 strips of (128, S) per core
ROWS = B * DSH  # 1024 rows of the flattened per-core input
SPAD = S + K - 1  # 4102
NXBUF = int(os.environ.get("KNXBUF", 4))  # x buffering depth
NACC = int(os.environ.get("KNACC", 3))  # acc buffering depth

# Column split per strip (sums to S): TensorE | GpSimd | VectorE
CPE = int(os.environ.get("KCPE", 1536))  # multiple of 512
CPOOL = int(os.environ.get("KCPOOL", 0))
CDVE = S - CPE - CPOOL
NCHUNK = CPE // 512  # PSUM chunks per strip
NPSUMB = int(os.environ.get("KNPSUMB", 8))  # PSUM bank buffers (of 8)

LAST_RESULT = None  # BassKernelResults of the most recent run (for test.py)

_NC_CACHE = {}


def _build_dma_only(rounds=1, nxbuf=3):
    """Loads + immediate stores, no compute — measures the pure DMA floor."""
    import concourse.bass as bass
    import concourse.mybir as mybir

    f32 = mybir.dt.float32
    nc = bass.Bass()
    xpad = nc.declare_dram_parameter("xpad", [ROWS, SPAD], f32, isOutput=False)
    wb = nc.declare_dram_parameter("wb", [128, 8 * NGROUPS], f32, isOutput=False)
    wdiag = nc.declare_dram_parameter(
        "wdiag", [128, NGROUPS * K * 128], f32, isOutput=False
    )
    out = nc.declare_dram_parameter("out", [ROWS, S], f32, isOutput=True)
    total = rounds * NSTRIPS
    with (
        nc.semaphore("load_sem") as load_sem,
        nc.semaphore("store_sem") as store_sem,
        nc.sbuf_tensor("x_sb", [128, nxbuf * SPAD], f32) as x_sb,
    ):
        with nc.Block() as block:

            @block.sync
            def _(sync):
                for t_ in range(total):
                    s_ = t_ % NSTRIPS
                    if t_ >= nxbuf:
                        sync.wait_ge(store_sem, 16 * (t_ - nxbuf + 1))
                    xb = (t_ % nxbuf) * SPAD
                    sync.dma_start(
                        out=x_sb[:, xb : xb + SPAD],
                        in_=xpad[128 * s_ : 128 * (s_ + 1), :],
                    ).then_inc(load_sem, 16)

            @block.scalar
            def _(scalar):
                for t_ in range(total):
                    s_ = t_ % NSTRIPS
                    xb = (t_ % nxbuf) * SPAD
                    scalar.wait_ge(load_sem, 16 * (t_ + 1))
                    scalar.dma_start(
                        out=out[128 * s_ : 128 * (s_ + 1), :],
                        in_=x_sb[:, xb : xb + S],
                    ).then_inc(store_sem, 16)

    return nc


def _build_bass(rounds=1, cpe=None, cpool=None, nxbuf=None, nacc=None,
                lsplit=None, ssplit=None, npsumb=None, dvesplit=None,
                tapmajor=None):
    import concourse.bass as bass
    import concourse.mybir as mybir

    CPE = cpe if cpe is not None else globals()["CPE"]
    CPOOL = cpool if cpool is not None else globals()["CPOOL"]
    NXBUF = nxbuf if nxbuf is not None else globals()["NXBUF"]
    NACC = nacc if nacc is not None else globals()["NACC"]
    NPSUMB = npsumb if npsumb is not None else globals()["NPSUMB"]
    CDVE = S - CPE - CPOOL
    NCHUNK = CPE // 512

    f32 = mybir.dt.float32
    mult = mybir.AluOpType.mult
    add = mybir.AluOpType.add
    ident = mybir.ActivationFunctionType.Identity

    nc = bass.Bass()
    xpad = nc.declare_dram_parameter("xpad", [ROWS, SPAD], f32, isOutput=False)
    wb = nc.declare_dram_parameter("wb", [128, 8 * NGROUPS], f32, isOutput=False)
    wdiag = nc.declare_dram_parameter(
        "wdiag", [128, NGROUPS * K * 128], f32, isOutput=False
    )
    out = nc.declare_dram_parameter("out", [ROWS, S], f32, isOutput=True)

    total = rounds * NSTRIPS

    with (
        nc.semaphore("load_sem") as load_sem,
        nc.semaphore("wb_sem") as wb_sem,
        nc.semaphore("wd_sem") as wd_sem,
        nc.semaphore("dve_sem") as dve_sem,
        nc.semaphore("pool_sem") as pool_sem,
        nc.semaphore("mm_sem") as mm_sem,
        nc.semaphore("act_sem") as act_sem,
        nc.semaphore("store_sem") as store_sem,
        nc.sbuf_tensor("wb_sb", [128, 8 * NGROUPS], f32) as wb_sb,
        nc.sbuf_tensor("wd_sb", [128, NGROUPS * K * 128], f32) as wd_sb,
        nc.sbuf_tensor("x_sb", [128, NXBUF * SPAD], f32) as x_sb,
        nc.sbuf_tensor("acc_sb", [128, NACC * S], f32) as acc_sb,
        nc.psum_tensor("ps", [128, NPSUMB * 512], f32) as ps,
    ):
        with nc.Block() as block:

            # Split point of the per-strip load: PE region first, rest second.
            _lsp = lsplit if lsplit is not None else bool(int(os.environ.get("KLSPLIT", "1")))
            LSPLIT = (min(CPE + K - 1, SPAD) if NCHUNK else 0) if _lsp else 0

            @block.sync
            def _(sync):
                sync.dma_start(out=wb_sb[:, :], in_=wb[:, :]).then_inc(wb_sem, 16)
                for t_ in range(total):
                    s_ = t_ % NSTRIPS
                    if t_ >= NXBUF:
                        # x buffer reuse: strip t_-NXBUF fully consumed
                        v = t_ - NXBUF + 1
                        if CDVE:
                            sync.wait_ge(dve_sem, v)
                        if CPOOL:
                            sync.wait_ge(pool_sem, v)
                        if NCHUNK:
                            sync.wait_ge(mm_sem, v * NCHUNK)
                    xb = (t_ % NXBUF) * SPAD
                    if LSPLIT:
                        sync.dma_start(
                            out=x_sb[:, xb : xb + LSPLIT],
                            in_=xpad[128 * s_ : 128 * (s_ + 1), 0:LSPLIT],
                        ).then_inc(load_sem, 16)
                        sync.dma_start(
                            out=x_sb[:, xb + LSPLIT : xb + SPAD],
                            in_=xpad[128 * s_ : 128 * (s_ + 1), LSPLIT:SPAD],
                        ).then_inc(load_sem, 16)
                    else:
                        sync.dma_start(
                            out=x_sb[:, xb : xb + SPAD],
                            in_=xpad[128 * s_ : 128 * (s_ + 1), :],
                        ).then_inc(load_sem, 16)

            LPS = 32 if LSPLIT else 16  # load_sem increments per strip
            _ssp = ssplit if ssplit is not None else bool(int(os.environ.get("KSSPLIT", "1")))
            SSPLIT = bool(NCHUNK and CDVE) and _ssp
            SPS = 32 if SSPLIT else 16  # store_sem increments per strip

            _tmaj = tapmajor if tapmajor is not None else bool(
                int(os.environ.get("KTAPMAJOR", "1"))
            )

            if NCHUNK:

                @block.tensor
                def _(tensor):
                    tensor.wait_ge(wb_sem, 16)
                    tensor.wait_ge(wd_sem, 16)
                    for t_ in range(total):
                        s_ = t_ % NSTRIPS
                        g = s_ % NGROUPS
                        xb = (t_ % NXBUF) * SPAD
                        tensor.wait_ge(load_sem, LPS * t_ + 16)
                        if _tmaj:
                            # tap-major: all chunks per tap — consecutive
                            # matmuls share the stationary diag
                            for c_ in range(NCHUNK):
                                ci = t_ * NCHUNK + c_
                                if ci >= NPSUMB:
                                    tensor.wait_ge(act_sem, ci - NPSUMB + 1)
                            for j in range(K):
                                for c_ in range(NCHUNK):
                                    ci = t_ * NCHUNK + c_
                                    bank = ci % NPSUMB
                                    c0 = 512 * c_
                                    ins = tensor.matmul(
                                        ps[:, 512 * bank : 512 * (bank + 1)],
                                        wd_sb[
                                            :,
                                            (K * g + j) * 128 : (K * g + j + 1) * 128,
                                        ],
                                        x_sb[:, xb + c0 + j : xb + c0 + j + 512],
                                        start=(j == 0),
                                        stop=(j == K - 1),
                                    )
                                    if j == K - 1:
                                        ins.then_inc(mm_sem, 1)
                        else:
                            for c_ in range(NCHUNK):
                                ci = t_ * NCHUNK + c_
                                bank = ci % NPSUMB
                                if ci >= NPSUMB:
                                    # PSUM bank reuse: ACT copied it out
                                    tensor.wait_ge(act_sem, ci - NPSUMB + 1)
                                c0 = 512 * c_
                                for j in range(K):
                                    ins = tensor.matmul(
                                        ps[:, 512 * bank : 512 * (bank + 1)],
                                        wd_sb[
                                            :,
                                            (K * g + j) * 128 : (K * g + j + 1) * 128,
                                        ],
                                        x_sb[:, xb + c0 + j : xb + c0 + j + 512],
                                        start=(j == 0),
                                        stop=(j == K - 1),
                                    )
                                ins.then_inc(mm_sem, 1)

            _dvs = dvesplit if dvesplit is not None else bool(
                int(os.environ.get("KDVESPLIT", "0"))
            )

            @block.vector
            def _(vector):
                if not CDVE:
                    return
                vector.wait_ge(wb_sem, 16)
                off = CPE + CPOOL
                for t_ in range(total):
                    s_ = t_ % NSTRIPS
                    g = s_ % NGROUPS
                    xb = (t_ % NXBUF) * SPAD
                    ab = (t_ % NACC) * S
                    vector.wait_ge(load_sem, LPS * (t_ + 1))
                    if t_ >= NACC:
                        vector.wait_ge(store_sem, SPS * (t_ - NACC + 1))
                    # halves: two independent chains interleaved so each op's
                    # pipe drain overlaps the other chain's fill (if the HW
                    # honors dependencies for the drain)
                    if _dvs:
                        h = CDVE // 2
                        parts = [(off, h), (off + h, CDVE - h)]
                    else:
                        parts = [(off, CDVE)]
                    for p0, pw in parts:
                        vector.tensor_scalar(
                            out=acc_sb[:, ab + p0 : ab + p0 + pw],
                            in0=x_sb[:, xb + p0 : xb + p0 + pw],
                            scalar1=wb_sb[:, 8 * g : 8 * g + 1],
                            scalar2=wb_sb[:, 8 * g + 7 : 8 * g + 8],
                            op0=mult,
                            op1=add,
                        )
                    for j in range(1, K):
                        for pi, (p0, pw) in enumerate(parts):
                            ins = vector.scalar_tensor_tensor(
                                out=acc_sb[:, ab + p0 : ab + p0 + pw],
                                in0=x_sb[:, xb + p0 + j : xb + p0 + j + pw],
                                scalar=wb_sb[:, 8 * g + j : 8 * g + j + 1],
                                in1=acc_sb[:, ab + p0 : ab + p0 + pw],
                                op0=mult,
                                op1=add,
                            )
                            if j == K - 1 and pi == len(parts) - 1:
                                ins.then_inc(dve_sem, 1)

            @block.gpsimd
            def _(gpsimd):
                if not CPOOL:
                    return
                gpsimd.wait_ge(wb_sem, 16)
                off = CPE
                for t_ in range(total):
                    s_ = t_ % NSTRIPS
                    g = s_ % NGROUPS
                    xb = (t_ % NXBUF) * SPAD
                    ab = (t_ % NACC) * S
                    gpsimd.wait_ge(load_sem, LPS * (t_ + 1))
                    if t_ >= NACC:
                        gpsimd.wait_ge(store_sem, SPS * (t_ - NACC + 1))
                    gpsimd.tensor_scalar(
                        out=acc_sb[:, ab + off : ab + off + CPOOL],
                        in0=x_sb[:, xb + off : xb + off + CPOOL],
                        scalar1=wb_sb[:, 8 * g : 8 * g + 1],
                        scalar2=wb_sb[:, 8 * g + 7 : 8 * g + 8],
                        op0=mult,
                        op1=add,
                    )
                    for j in range(1, K):
                        ins = gpsimd.scalar_tensor_tensor(
                            out=acc_sb[:, ab + off : ab + off + CPOOL],
                            in0=x_sb[:, xb + off + j : xb + off + j + CPOOL],
                            scalar=wb_sb[:, 8 * g + j : 8 * g + j + 1],
                            in1=acc_sb[:, ab + off : ab + off + CPOOL],
                            op0=mult,
                            op1=add,
                        )
                        if j == K - 1:
                            ins.then_inc(pool_sem, 1)

            @block.scalar
            def _(scalar):
                scalar.dma_start(out=wd_sb[:, :], in_=wdiag[:, :]).then_inc(wd_sem, 16)
                for t_ in range(total):
                    s_ = t_ % NSTRIPS
                    g = s_ % NGROUPS
                    ab = (t_ % NACC) * S
                    if NCHUNK and t_ >= NACC:
                        scalar.wait_ge(store_sem, SPS * (t_ - NACC + 1))
                    for c_ in range(NCHUNK):
                        ci = t_ * NCHUNK + c_
                        bank = ci % NPSUMB
                        scalar.wait_ge(mm_sem, ci + 1)
                        scalar.activation(
                            out=acc_sb[:, ab + 512 * c_ : ab + 512 * (c_ + 1)],
                            in_=ps[:, 512 * bank : 512 * (bank + 1)],
                            func=ident,
                            bias=wb_sb[:, 8 * g + 7 : 8 * g + 8],
                            scale=1.0,
                        ).then_inc(act_sem, 1)
                    if SSPLIT:
                        # PE region ready (ACT program order covers the copies)
                        scalar.dma_start(
                            out=out[128 * s_ : 128 * (s_ + 1), 0:CPE],
                            in_=acc_sb[:, ab : ab + CPE],
                        ).then_inc(store_sem, 16)
                    if CDVE:
                        scalar.wait_ge(dve_sem, t_ + 1)
                    if CPOOL:
                        scalar.wait_ge(pool_sem, t_ + 1)
                    sc0 = CPE if SSPLIT else 0
                    scalar.dma_start(
                        out=out[128 * s_ : 128 * (s_ + 1), sc0:S],
                        in_=acc_sb[:, ab + sc0 : ab + S],
                    ).then_inc(store_sem, 16)

    return nc


def _build_bass_v3(rounds=1, cpe=1536, npsumb=8, lsplit=True, ssplit=True):
    """v3: DVE consumes strip PAIRS (same weight group) per op via 3D paged
    APs — halves DVE instruction count (and per-op DRAIN overhead)."""
    import concourse.bass as bass
    import concourse.mybir as mybir

    f32 = mybir.dt.float32
    mult = mybir.AluOpType.mult
    add = mybir.AluOpType.add
    ident = mybir.ActivationFunctionType.Identity

    CPE = cpe
    CDVE = S - CPE
    NCHUNK = CPE // 512
    NPSUMB = npsumb
    NXB = 4
    NAC = 4

    assert NSTRIPS % 4 == 0

    nc = bass.Bass()
    xpad = nc.declare_dram_parameter("xpad", [ROWS, SPAD], f32, isOutput=False)
    wb = nc.declare_dram_parameter("wb", [128, 8 * NGROUPS], f32, isOutput=False)
    wdiag = nc.declare_dram_parameter(
        "wdiag", [128, NGROUPS * K * 128], f32, isOutput=False
    )
    out = nc.declare_dram_parameter("out", [ROWS, S], f32, isOutput=True)

    total = rounds * NSTRIPS

    def pair_val(t_):
        # dve_sem value after the pair containing global strip t_ completes
        r_, s_ = divmod(t_, NSTRIPS)
        return r_ * (NSTRIPS // 2) + (s_ // 4) * 2 + (s_ % 4) % 2 + 1

    with (
        nc.semaphore("load_sem") as load_sem,
        nc.semaphore("wb_sem") as wb_sem,
        nc.semaphore("wd_sem") as wd_sem,
        nc.semaphore("dve_sem") as dve_sem,
        nc.semaphore("mm_sem") as mm_sem,
        nc.semaphore("act_sem") as act_sem,
        nc.semaphore("store_sem") as store_sem,
        nc.sbuf_tensor("wb_sb", [128, 8 * NGROUPS], f32) as wb_sb,
        nc.sbuf_tensor("wd_sb", [128, NGROUPS * K * 128], f32) as wd_sb,
        nc.sbuf_tensor("x_sb", [128, NXB * SPAD], f32) as x_sb,
        nc.sbuf_tensor("acc_sb", [128, NAC * S], f32) as acc_sb,
        nc.psum_tensor("ps", [128, NPSUMB * 512], f32) as ps,
    ):
        x4 = x_sb[:, :].rearrange("p (s n) -> p s n", s=NXB)
        a4 = acc_sb[:, :].rearrange("p (s n) -> p s n", s=NAC)

        with nc.Block() as block:
            LSPLIT = (min(CPE + K - 1, SPAD) if NCHUNK else 0) if lsplit else 0
            LPS = 32 if LSPLIT else 16
            SSPLIT = bool(NCHUNK and CDVE) and ssplit
            SPS = 32 if SSPLIT else 16

            @block.sync
            def _(sync):
                sync.dma_start(out=wb_sb[:, :], in_=wb[:, :]).then_inc(wb_sem, 16)
                for t_ in range(total):
                    s_ = t_ % NSTRIPS
                    if t_ >= NXB:
                        if CDVE:
                            sync.wait_ge(dve_sem, pair_val(t_ - NXB))
                        if NCHUNK:
                            sync.wait_ge(mm_sem, (t_ - NXB + 1) * NCHUNK)
                    xb = (t_ % NXB) * SPAD
                    if LSPLIT:
                        sync.dma_start(
                            out=x_sb[:, xb : xb + LSPLIT],
                            in_=xpad[128 * s_ : 128 * (s_ + 1), 0:LSPLIT],
                        ).then_inc(load_sem, 16)
                        sync.dma_start(
                            out=x_sb[:, xb + LSPLIT : xb + SPAD],
                            in_=xpad[128 * s_ : 128 * (s_ + 1), LSPLIT:SPAD],
                        ).then_inc(load_sem, 16)
                    else:
                        sync.dma_start(
                            out=x_sb[:, xb : xb + SPAD],
                            in_=xpad[128 * s_ : 128 * (s_ + 1), :],
                        ).then_inc(load_sem, 16)

            if NCHUNK:

                @block.tensor
                def _(tensor):
                    tensor.wait_ge(wb_sem, 16)
                    tensor.wait_ge(wd_sem, 16)
                    for t_ in range(total):
                        s_ = t_ % NSTRIPS
                        g = s_ % NGROUPS
                        xb = (t_ % NXB) * SPAD
                        tensor.wait_ge(load_sem, LPS * t_ + 16)
                        for c_ in range(NCHUNK):
                            ci = t_ * NCHUNK + c_
                            bank = ci % NPSUMB
                            if ci >= NPSUMB:
                                tensor.wait_ge(act_sem, ci - NPSUMB + 1)
                            c0 = 512 * c_
                            for j in range(K):
                                ins = tensor.matmul(
                                    ps[:, 512 * bank : 512 * (bank + 1)],
                                    wd_sb[:, (K * g + j) * 128 : (K * g + j + 1) * 128],
                                    x_sb[:, xb + c0 + j : xb + c0 + j + 512],
                                    start=(j == 0),
                                    stop=(j == K - 1),
                                )
                            ins.then_inc(mm_sem, 1)

            @block.vector
            def _(vector):
                if not CDVE:
                    return
                vector.wait_ge(wb_sem, 16)
                for t0 in range(0, total, 4):
                    for u in (0, 1):
                        a_ = t0 + u  # pair (a_, a_+2); same weight group
                        b_ = a_ + 2
                        g = (a_ % NSTRIPS) % NGROUPS
                        xs = a_ % NXB  # pages xs, xs+2
                        vector.wait_ge(load_sem, LPS * (b_ + 1))
                        if a_ >= NXB:
                            vector.wait_ge(store_sem, SPS * (b_ - NXB + 1))
                        x3 = x4[:, xs :: 2, :]
                        o3 = a4[:, xs :: 2, CPE:S]
                        vector.tensor_scalar(
                            out=o3,
                            in0=x3[:, :, CPE : CPE + CDVE],
                            scalar1=wb_sb[:, 8 * g : 8 * g + 1],
                            scalar2=wb_sb[:, 8 * g + 7 : 8 * g + 8],
                            op0=mult,
                            op1=add,
                        )
                        for j in range(1, K):
                            ins = vector.scalar_tensor_tensor(
                                out=o3,
                                in0=x3[:, :, CPE + j : CPE + j + CDVE],
                                scalar=wb_sb[:, 8 * g + j : 8 * g + j + 1],
                                in1=o3,
                                op0=mult,
                                op1=add,
                            )
                            if j == K - 1:
                                ins.then_inc(dve_sem, 1)

            @block.scalar
            def _(scalar):
                scalar.dma_start(out=wd_sb[:, :], in_=wdiag[:, :]).then_inc(wd_sem, 16)
                for t_ in range(total):
                    s_ = t_ % NSTRIPS
                    g = s_ % NGROUPS
                    ab = (t_ % NAC) * S
                    if NCHUNK and t_ >= NAC:
                        scalar.wait_ge(store_sem, SPS * (t_ - NAC + 1))
                    for c_ in range(NCHUNK):
                        ci = t_ * NCHUNK + c_
                        bank = ci % NPSUMB
                        scalar.wait_ge(mm_sem, ci + 1)
                        scalar.activation(
                            out=acc_sb[:, ab + 512 * c_ : ab + 512 * (c_ + 1)],
                            in_=ps[:, 512 * bank : 512 * (bank + 1)],
                            func=ident,
                            bias=wb_sb[:, 8 * g + 7 : 8 * g + 8],
                            scale=1.0,
                        ).then_inc(act_sem, 1)
                    if SSPLIT:
                        scalar.dma_start(
                            out=out[128 * s_ : 128 * (s_ + 1), 0:CPE],
                            in_=acc_sb[:, ab : ab + CPE],
                        ).then_inc(store_sem, 16)
                    if CDVE:
                        scalar.wait_ge(dve_sem, pair_val(t_))
                    sc0 = CPE if SSPLIT else 0
                    scalar.dma_start(
                        out=out[128 * s_ : 128 * (s_ + 1), sc0:S],
                        in_=acc_sb[:, ab + sc0 : ab + S],
                    ).then_inc(store_sem, 16)

    return nc


def _get_bass():
    if "nc" not in _NC_CACHE:
        if int(os.environ.get("KV3", "0")):
            _NC_CACHE["nc"] = _build_bass_v3()
        else:
            _NC_CACHE["nc"] = _build_bass()
    return _NC_CACHE["nc"]


def _prepare_in_maps(x, conv_state, W, b):
    x = np.asarray(x, dtype=np.float32)
    conv_state = np.asarray(conv_state, dtype=np.float32)
    W = np.asarray(W, dtype=np.float32)
    b = np.asarray(b, dtype=np.float32)

    in_maps = []
    for c in range(NCORES):
        sl = slice(c * DSH, (c + 1) * DSH)
        xt = x[:, :, sl].transpose(0, 2, 1)  # (B, DSH, S)
        xp = np.concatenate([conv_state[:, sl, :], xt], axis=2)  # (B, DSH, SPAD)
        xp = np.ascontiguousarray(xp).reshape(ROWS, SPAD)
        wc = W[sl, 0, :]  # (DSH, K)
        bc = b[sl]  # (DSH,)
        wbm = np.zeros((128, 8 * NGROUPS), dtype=np.float32)
        for g in range(NGROUPS):
            wbm[:, 8 * g : 8 * g + 7] = wc[128 * g : 128 * (g + 1)]
            wbm[:, 8 * g + 7] = bc[128 * g : 128 * (g + 1)]
        wdm = np.zeros((128, NGROUPS * K * 128), dtype=np.float32)
        idx = np.arange(128)
        for g in range(NGROUPS):
            for j in range(K):
                blk = K * g + j
                wdm[idx, blk * 128 + idx] = wc[128 * g : 128 * (g + 1), j]
        in_maps.append({"xpad": xp, "wb": wbm, "wdiag": wdm})
    return in_maps


def kernel(x, conv_state, W, b):
    global LAST_RESULT
    from concourse.bass_utils import run_bass_kernel_spmd

    x = np.asarray(x, dtype=np.float32)
    in_maps = _prepare_in_maps(x, conv_state, W, b)

    nc = _get_bass()
    trace = bool(os.environ.get("KBENCH_TRACE"))
    res = run_bass_kernel_spmd(nc, in_maps, list(range(NCORES)), trace=trace)
    LAST_RESULT = res

    shards = []
    for c in range(NCORES):
        o = res.results[c]["out"].reshape(B, DSH, S).transpose(0, 2, 1)  # (B,S,DSH)
        shards.append(o)
    out = np.concatenate(shards, axis=2)  # (B, S, D)

    new_state = np.ascontiguousarray(x[:, S - (K - 1) :, :].transpose(0, 2, 1))
    return out, new_state
